# revision 40
# baseline (speedup 1.0000x reference)
"""Trainium2 Bass kernel for nn_CrossAttention (B=8, S1=S2=2048, D=512, single head).

Sharding: batch dim B=8 across the 8 NeuronCores (data parallel). Each core runs
the full cross-attention for one batch element:
    q = RoPE(h1 @ Wq.T + bq); k = RoPE(h2 @ Wk.T + bk); v = h2 @ Wv.T + bv
    out = softmax(q k^T / sqrt(D)) v @ Wo.T + bo

Design notes:
  - All matmuls in bf16 (fp32 PSUM accumulation): rel_l2 vs fp32 reference ~6e-3.
  - Scores are computed TRANSPOSED (S^T[k,q]) so the probability matrix feeds the
    PV matmul directly as the moving operand - no P transposes.
  - Softmax skips max-subtraction (energies are ~N(0,1), |e| < 8, exp is safe in
    fp32) so no partition-dim max is needed.
  - Column sums (denominators) via a ones-vector matmul accumulated in PSUM; the
    sum row [1,512] is broadcast to 128 partitions with a K=1 fp32 matmul, the
    reciprocal runs wide [128,512], and normalization lands on O^T (free dim = q).
  - h1/h2 are transposed on-chip via PE transpose (fp32 DMA transpose is not
    supported; tiles are cast to bf16 first so the transpose runs 1 cyc/row);
    weights / RoPE tables are pre-transposed on host (replicated, tiny), bf16.
  - Prologue is interleaved per 512-row slice (DMA -> transpose -> project ->
    RoPE) with per-slice SBUF tiles so nothing serializes on whole-tensor deps,
    and DMAs are emitted in true dependency order.
"""

import math
import sys

import numpy as np

for _p in ("/opt/trn_rl_repo",):
    if _p not in sys.path:
        sys.path.insert(0, _p)

import ml_dtypes

BF16 = ml_dtypes.bfloat16

S = 2048
D = 512
P = 128
B = 8
NB = S // P      # 16 key blocks of 128
DC = D // P      # 4 d-chunks of 128
EC = D // P      # 4 e-chunks (contraction for projections)
QW = 512         # tile width (free dim per matmul)
QT = S // QW     # 4 q tiles
SB = QW // P     # 4 s-blocks per q tile
NS = S // QW     # 4 s-slices for the prologue
SCALE = 1.0 / math.sqrt(D)

_compiled = None


def _build():
    import concourse.bass as bass  # noqa: F401
    import concourse.mybir as mybir
    import concourse.tile as tile
    from concourse import bacc

    f32 = mybir.dt.float32
    bf16 = mybir.dt.bfloat16
    Alu = mybir.AluOpType
    Act = mybir.ActivationFunctionType

    nc = bacc.Bacc("TRN2", target_bir_lowering=False, debug=False, num_devices=B)

    # h1/h2 arrive pre-transposed ([D, S], feature dim outer) and pre-cast to
    # bf16 on host: fp32 cannot DMA-transpose and the matmuls are bf16 anyway
    h1t_d = nc.dram_tensor("h1t", [D, S], bf16, kind="ExternalInput").ap()
    h2t_d = nc.dram_tensor("h2t", [D, S], bf16, kind="ExternalInput").ap()
    w_dram = {
        name: nc.dram_tensor(f"{name}_t", [D, D], bf16, kind="ExternalInput").ap()
        for name in ("wq", "wk", "wv", "wo")
    }
    cos_t = nc.dram_tensor("cos_t", [D, S], bf16, kind="ExternalInput").ap()
    sin_t = nc.dram_tensor("sin_t", [D, S], bf16, kind="ExternalInput").ap()
    bq_c = nc.dram_tensor("bq_c", [P, DC], f32, kind="ExternalInput").ap()
    bk_c = nc.dram_tensor("bk_c", [P, DC], f32, kind="ExternalInput").ap()
    # bo_b holds bo_eff = bo + Wo @ bv (bv folded through the value path on host)
    bo_b = nc.dram_tensor("bo_b", [P, D], f32, kind="ExternalInput").ap()
    ident_d = nc.dram_tensor("ident", [P, P], f32, kind="ExternalInput").ap()
    out = nc.dram_tensor("out", [S, D], f32, kind="ExternalOutput").ap()

    with tile.TileContext(nc) as tc:
        from contextlib import ExitStack

        with ExitStack() as ctx:
            singles = ctx.enter_context(tc.tile_pool(name="singles", bufs=1))

            def load_w(name):
                # one dma_start per weight: DMA *issue* costs ~0.7us on the
                # sequencer, so fewer+bigger transfers win at the front
                t = singles.tile([P, EC, D], bf16, tag=f"w_{name}")
                nc.sync.dma_start(
                    out=t, in_=w_dram[name].rearrange("(c p) d -> p c d", p=P)
                )
                return t

            # --- persistent tiles (DMAs emitted in dependency order) ---------
            w_sb = {}
            w_sb["wk"] = load_w("wk")
            bk_sb = singles.tile([P, DC], f32, tag="bk")
            nc.sync.dma_start(out=bk_sb, in_=bk_c)

            # per-slice persistent tensors (fine-grained dependencies)
            kt_p = [
                singles.tile([P, DC, QW], bf16, tag=f"kt{i}", name=f"kt{i}")
                for i in range(NS)
            ]
            qt_p = [
                singles.tile([P, DC, QW], bf16, tag=f"qt{i}", name=f"qt{i}")
                for i in range(NS)
            ]
            v_p = [
                singles.tile([P, SB, QW], bf16, tag=f"v{i}", name=f"v{i}")
                for i in range(NS)
            ]

            cos_sb = singles.tile([P, DC, S], bf16, tag="cos")
            sin_sb = singles.tile([P, DC, S], bf16, tag="sin")

            # ---------------- Phase A: k/v projections + RoPE ----------------
            htp = ctx.enter_context(tc.tile_pool(name="ht", bufs=3))
            ptmp = ctx.enter_context(tc.tile_pool(name="ptmp", bufs=3))
            h1t_r = h1t_d.rearrange("(c p) s -> p c s", p=P)
            h2t_r = h2t_d.rearrange("(c p) s -> p c s", p=P)

            if True:  # keep indentation structure

                def load_slice(ht_r, s2):
                    # [P, EC, QW] bf16 tile holding h^T columns [s2*QW,(s2+1)*QW)
                    ht = htp.tile([P, EC, QW], bf16, tag="ht")
                    nc.sync.dma_start(
                        out=ht, in_=ht_r[:, :, s2 * QW : (s2 + 1) * QW]
                    )
                    return ht

                def project_rope(ht, wname, b_sb, dst, s2, pp_pool, pp_tag):
                    # dst[:, dc, :] = RoPE(W @ h^T + b) for columns of slice s2
                    sl = slice(s2 * QW, (s2 + 1) * QW)
                    for pair in range(2):
                        dc0, dc2 = pair, pair + 2
                        pp = pp_pool.tile([P, 2, QW], f32, tag=pp_tag, name="pp")
                        for half, dc in ((0, dc0), (1, dc2)):
                            for ec in range(EC):
                                nc.tensor.matmul(
                                    pp[:, half, :],
                                    lhsT=w_sb[wname][:, ec, dc * P : (dc + 1) * P],
                                    rhs=ht[:, ec, :],
                                    start=(ec == 0),
                                    stop=(ec == EC - 1),
                                )
                        # rope: out[d<256] = x0*cos0 - x2*sin0
                        #       out[d>=256] = x2*cos2 + x0*sin2
                        # (bias folds into the STT's scalar add; the combines
                        # run on the otherwise-idle GpSimd engine)
                        t0 = ptmp.tile([P, QW], f32, tag="rope0")
                        nc.vector.scalar_tensor_tensor(
                            t0,
                            in0=pp[:, 0, :],
                            scalar=b_sb[:, dc0 : dc0 + 1],
                            in1=cos_sb[:, dc0, sl],
                            op0=Alu.add,
                            op1=Alu.mult,
                        )
                        t1 = ptmp.tile([P, QW], f32, tag="rope1")
                        nc.vector.scalar_tensor_tensor(
                            t1,
                            in0=pp[:, 1, :],
                            scalar=b_sb[:, dc2 : dc2 + 1],
                            in1=sin_sb[:, dc0, sl],
                            op0=Alu.add,
                            op1=Alu.mult,
                        )
                        nc.gpsimd.tensor_tensor(dst[:, dc0, :], t0, t1, Alu.subtract)
                        t2 = ptmp.tile([P, QW], f32, tag="rope0")
                        nc.vector.scalar_tensor_tensor(
                            t2,
                            in0=pp[:, 1, :],
                            scalar=b_sb[:, dc2 : dc2 + 1],
                            in1=cos_sb[:, dc2, sl],
                            op0=Alu.add,
                            op1=Alu.mult,
                        )
                        t3 = ptmp.tile([P, QW], f32, tag="rope1")
                        nc.vector.scalar_tensor_tensor(
                            t3,
                            in0=pp[:, 0, :],
                            scalar=b_sb[:, dc0 : dc0 + 1],
                            in1=sin_sb[:, dc2, sl],
                            op0=Alu.add,
                            op1=Alu.mult,
                        )
                        nc.gpsimd.tensor_tensor(dst[:, dc2, :], t2, t3, Alu.add)

                def project_v(ht, s2, psA):
                    # bv is folded into bo on host (bo_eff = bo + Wo @ bv), so
                    # this is a plain PSUM->SBUF cast on the idle ACT engine
                    for j in range(SB):
                        vp = psA.tile([P, QW], f32, tag="vp")
                        for ec in range(EC):
                            nc.tensor.matmul(
                                vp,
                                lhsT=ht[:, ec, j * P : (j + 1) * P],
                                rhs=w_sb["wv"][:, ec, :],
                                start=(ec == 0),
                                stop=(ec == EC - 1),
                            )
                        nc.scalar.copy(v_p[s2][:, j, :], vp)

            with tc.tile_pool(name="psumA", bufs=2, space="PSUM") as psA:
                cos_r = cos_t.rearrange("(c p) s -> p c s", p=P)
                sin_r = sin_t.rearrange("(c p) s -> p c s", p=P)
                for s2 in range(NS):
                    ht = load_slice(h2t_r, s2)
                    # interleave the RoPE-table chunk transfers with the h2t
                    # slices: slice RoPE pair0 needs chunks (0,2), pair1 (1,3)
                    if s2 == 0:
                        # tables issue from the (idle) gpsimd sequencer so they
                        # don't delay issue of the critical-path sync DMAs
                        nc.gpsimd.dma_start(out=cos_sb, in_=cos_r)
                        nc.gpsimd.dma_start(out=sin_sb, in_=sin_r)
                        w_sb["wv"] = load_w("wv")
                    project_rope(ht, "wk", bk_sb, kt_p[s2], s2, psA, "pp")
                    if s2 == 2:
                        # q slice 0 before the last k slice: its RoPE clears
                        # the DVE before attention needs qt_p[0]; attention
                        # only reads kt_p[3] ~25us into its kb loop
                        ht0 = load_slice(h1t_r, 0)
                        project_rope(ht0, "wq", bq_sb, qt_p[0], 0, psA, "pp")
                    project_v(ht, s2, psA)
                    if s2 == 0:
                        w_sb["wq"] = load_w("wq")
                        bq_sb = singles.tile([P, DC], f32, tag="bq")
                        nc.gpsimd.dma_start(out=bq_sb, in_=bq_c)
                        ident = singles.tile([P, P], f32, tag="ident")
                        nc.gpsimd.dma_start(out=ident, in_=ident_d)
                        ones_bf = singles.tile([P, 1], bf16, tag="ones_bf")
                        nc.vector.memset(ones_bf, 1.0)
                    if s2 == 1:
                        w_sb["wo"] = load_w("wo")
                        bo_sb = singles.tile([P, D], f32, tag="bo")
                        nc.gpsimd.dma_start(out=bo_sb, in_=bo_b)

            # ---------------- Phase B: attention -----------------------------
            # PSUM budget (8 banks): st 3 + ot 2 + cs 1 + finpp 2.
            # PV accumulates in TWO passes of 2 d-chunks each (pt tiles stay
            # resident) so that 2 banks ("finpp") are free for the NEXT q
            # slice's projection, which runs in the shadow of this q tile's
            # attention.
            with tc.tile_pool(name="ptpool", bufs=NB + 2) as ptp, tc.tile_pool(
                name="otsb", bufs=2
            ) as otp, tc.tile_pool(name="outst", bufs=3) as outp, tc.tile_pool(
                name="psum_st", bufs=3, space="PSUM"
            ) as ps_st, tc.tile_pool(
                name="psum_ot", bufs=1, space="PSUM"
            ) as ps_ot, tc.tile_pool(
                name="psum_cs", bufs=1, space="PSUM"
            ) as ps_cs, tc.tile_pool(name="psum_fin", bufs=1, space="PSUM") as ps_fin:
                for qt in range(QT):
                    if qt + 1 < QT:
                        # project+RoPE the next q slice during this attention
                        htn = load_slice(h1t_r, qt + 1)
                        project_rope(
                            htn, "wq", bq_sb, qt_p[qt + 1], qt + 1, ps_fin, "finpp"
                        )

                    cs = ps_cs.tile([1, QW], f32, tag="cs")
                    ot_sb = otp.tile([P, DC, QW], bf16, tag="ot_sb")
                    pts = []

                    # pass 1: S^T + exp + colsums + PV dc 0,1; PV(kb-1) is
                    # emitted after S^T(kb) so the PE never head-of-line
                    # blocks on exp(kb)
                    ot01 = ps_ot.tile([P, 2, QW], f32, tag="ot", name="ot01")
                    for kb in range(NB):
                        st = ps_st.tile([P, QW], f32, tag="st")
                        for dc in range(DC):
                            nc.tensor.matmul(
                                st,
                                lhsT=kt_p[kb // SB][:, dc, (kb % SB) * P : (kb % SB + 1) * P],
                                rhs=qt_p[qt][:, dc, :],
                                start=(dc == 0),
                                stop=(dc == DC - 1),
                            )
                        pt = ptp.tile([P, QW], bf16, tag="pt")
                        nc.scalar.activation(pt, st, Act.Exp, scale=SCALE)
                        pts.append(pt)
                        if kb > 0:
                            _emit_pv(nc, v_p, ones_bf, pts[kb - 1], ot01, cs, kb - 1, (0, 1))
                    _emit_pv(nc, v_p, ones_bf, pts[NB - 1], ot01, cs, NB - 1, (0, 1))
                    for dc in range(2):
                        nc.any.tensor_copy(out=ot_sb[:, dc, :], in_=ot01[:, dc, :])

                    # pass 2: PV dc 2,3 from the resident pt tiles
                    ot23 = ps_ot.tile([P, 2, QW], f32, tag="ot", name="ot23")
                    for kb in range(NB):
                        for i, dc in enumerate((2, 3)):
                            nc.tensor.matmul(
                                ot23[:, i, :],
                                lhsT=v_p[kb // SB][:, kb % SB, dc * P : (dc + 1) * P],
                                rhs=pts[kb],
                                start=(kb == 0),
                                stop=(kb == NB - 1),
                            )
                    for i, dc in enumerate((2, 3)):
                        nc.any.tensor_copy(out=ot_sb[:, dc, :], in_=ot23[:, i, :])

                    # denominators: DVE reciprocal is per-lane-serial, so get the
                    # colsums onto PARTITIONS first (4 tiny PE transposes of
                    # [1,128] slices), then reciprocal on [128,4] is ~free.
                    cs_row = outp.tile([1, QW], f32, tag="cs_row")
                    nc.any.tensor_copy(out=cs_row, in_=cs)
                    r4 = outp.tile([P, SB], f32, tag="r4")
                    for sb in range(SB):
                        tpr = ps_cs.tile([P, 1], f32, tag="cs", name="tpr")
                        nc.tensor.transpose(
                            tpr, cs_row[0:1, sb * P : (sb + 1) * P], ident[0:1, 0:1]
                        )
                        nc.any.tensor_copy(out=r4[:, sb : sb + 1], in_=tpr)
                    r4r = outp.tile([P, SB], f32, tag="r4r")
                    nc.vector.reciprocal(r4r, r4)

                    # final projection back to natural [s, d] layout; fused
                    # (fp * r) + bo in one DVE op
                    for sb in range(SB):
                        fpt = ps_fin.tile([P, 2, QW], f32, tag="finpp", name="fpt")
                        fp = fpt[:, 0, :]
                        for dc in range(DC):
                            nc.tensor.matmul(
                                fp,
                                lhsT=ot_sb[:, dc, sb * P : (sb + 1) * P],
                                rhs=w_sb["wo"][:, dc, :],
                                start=(dc == 0),
                                stop=(dc == DC - 1),
                            )
                        o_sb = outp.tile([P, D], f32, tag="ostage")
                        nc.vector.scalar_tensor_tensor(
                            o_sb,
                            in0=fp,
                            scalar=r4r[:, sb : sb + 1],
                            in1=bo_sb,
                            op0=Alu.mult,
                            op1=Alu.add,
                        )
                        row0 = (qt * SB + sb) * P
                        nc.sync.dma_start(out=out[row0 : row0 + P, :], in_=o_sb)

    nc.compile()
    return nc


def _emit_pv(nc, v_p, ones_bf, pt, ot, cs, kb, dcs):
    nc.tensor.matmul(
        cs, lhsT=ones_bf, rhs=pt, start=(kb == 0), stop=(kb == NB - 1)
    )
    for i, dc in enumerate(dcs):
        nc.tensor.matmul(
            ot[:, i, :],
            lhsT=v_p[kb // SB][:, kb % SB, dc * P : (dc + 1) * P],
            rhs=pt,
            start=(kb == 0),
            stop=(kb == NB - 1),
        )


def _get_compiled():
    global _compiled
    if _compiled is None:
        _compiled = _build()
    return _compiled


def _host_tables():
    half = D // 2
    inv_freq = 1.0 / (10000.0 ** (np.arange(half, dtype=np.float32) / half))
    t = np.arange(S, dtype=np.float32)
    freqs = np.outer(t, inv_freq)
    emb = np.concatenate([freqs, freqs], axis=-1)  # [S, D]
    cos_t = np.ascontiguousarray(np.cos(emb).T).astype(BF16)  # [D, S]
    sin_t = np.ascontiguousarray(np.sin(emb).T).astype(BF16)
    return cos_t, sin_t


def make_in_maps(**inputs):
    cos_t, sin_t = _host_tables()
    shared = {
        "cos_t": cos_t,
        "sin_t": sin_t,
        "wq_t": np.ascontiguousarray(np.asarray(inputs["Wq"], np.float32).T).astype(BF16),
        "wk_t": np.ascontiguousarray(np.asarray(inputs["Wk"], np.float32).T).astype(BF16),
        "wv_t": np.ascontiguousarray(np.asarray(inputs["Wv"], np.float32).T).astype(BF16),
        "wo_t": np.ascontiguousarray(np.asarray(inputs["Wo"], np.float32).T).astype(BF16),
        "bq_c": np.ascontiguousarray(np.asarray(inputs["bq"], np.float32).reshape(DC, P).T),
        "bk_c": np.ascontiguousarray(np.asarray(inputs["bk"], np.float32).reshape(DC, P).T),
        # bv contributes bv @ Wo.T to every output row - fold it into bo
        "bo_b": np.ascontiguousarray(
            np.broadcast_to(
                np.asarray(inputs["bo"], np.float32)
                + np.asarray(inputs["Wo"], np.float32)
                @ np.asarray(inputs["bv"], np.float32),
                (P, D),
            )
        ),
        "ident": np.eye(P, dtype=np.float32),
    }
    h1 = np.asarray(inputs["h1"], np.float32)
    h2 = np.asarray(inputs["h2"], np.float32)
    return [
        dict(
            shared,
            h1t=np.ascontiguousarray(h1[core].T).astype(BF16),
            h2t=np.ascontiguousarray(h2[core].T).astype(BF16),
        )
        for core in range(B)
    ]


def _install_ntff_hook():
    """The agent image's antenv lacks axon_hooks; rebuild the NTFF profile hook
    from libaxon_pjrt.so (mirrors trn_agent_boot._ntff_profile_via_ctypes)."""
    try:
        from antenv.axon_hooks import get_axon_ntff_profile_hook  # noqa: F401

        return
    except ImportError:
        pass
    import contextlib
    import ctypes
    import types

    so_path = "/opt/axon/libaxon_pjrt.so"
    try:
        lib = ctypes.CDLL(so_path)
    except OSError:
        return
    if not hasattr(lib, "axon_start_nrt_profile"):
        return
    lib.axon_start_nrt_profile.argtypes = [
        ctypes.POINTER(ctypes.c_int64),
        ctypes.c_size_t,
    ]
    lib.axon_start_nrt_profile.restype = ctypes.c_int64
    lib.axon_stop_nrt_profile.argtypes = [ctypes.c_char_p]
    lib.axon_stop_nrt_profile.restype = ctypes.c_int64

    @contextlib.contextmanager
    def _hook(output_dir, device_ids):
        import jax

        jax.devices()
        if device_ids:
            ids = (ctypes.c_int64 * len(device_ids))(*device_ids)
            rc = lib.axon_start_nrt_profile(ids, len(device_ids))
        else:
            rc = lib.axon_start_nrt_profile(None, 0)
        if rc != 0:
            raise RuntimeError(f"axon_start_nrt_profile rc={rc}")
        try:
            yield
        finally:
            n = lib.axon_stop_nrt_profile(str(output_dir).encode())
            print(f"ntff profile: {n} file(s) written to {output_dir}")

    import antenv

    mod = types.ModuleType("antenv.axon_hooks")
    mod.get_axon_ntff_profile_hook = lambda: _hook
    mod.set_axon_ntff_profile_hook = lambda h: None
    sys.modules["antenv.axon_hooks"] = mod
    antenv.axon_hooks = mod


def run(trace=False, tmpdir=None, trace_cores=None, **inputs):
    from concourse.bass_utils import run_bass_kernel_spmd

    if trace:
        _install_ntff_hook()
    nc = _get_compiled()
    in_maps = make_in_maps(**inputs)
    kwargs = {}
    if tmpdir is not None:
        kwargs["tmpdir"] = tmpdir
    if trace_cores is not None:
        kwargs["trace_cores"] = trace_cores
    res = run_bass_kernel_spmd(
        nc, in_maps, core_ids=list(range(B)), trace=trace, **kwargs
    )
    out = np.stack([res.results[i]["out"] for i in range(B)]).astype(np.float32)
    return out, res


def kernel(**inputs):
    out, _ = run(trace=False, **inputs)
    return out


# revision 44
# speedup vs baseline: 1.0267x; 1.0267x over previous
"""Trainium2 Bass kernel for nn_CrossAttention (B=8, S1=S2=2048, D=512, single head).

Sharding: batch dim B=8 across the 8 NeuronCores (data parallel). Each core runs
the full cross-attention for one batch element:
    q = RoPE(h1 @ Wq.T + bq); k = RoPE(h2 @ Wk.T + bk); v = h2 @ Wv.T + bv
    out = softmax(q k^T / sqrt(D)) v @ Wo.T + bo

Design notes:
  - All matmuls in bf16 (fp32 PSUM accumulation): rel_l2 vs fp32 reference ~6e-3.
  - Scores are computed TRANSPOSED (S^T[k,q]) so the probability matrix feeds the
    PV matmul directly as the moving operand - no P transposes.
  - Softmax skips max-subtraction (energies are ~N(0,1), |e| < 8, exp is safe in
    fp32) so no partition-dim max is needed.
  - Column sums (denominators) via a ones-vector matmul accumulated in PSUM; the
    sum row [1,512] is broadcast to 128 partitions with a K=1 fp32 matmul, the
    reciprocal runs wide [128,512], and normalization lands on O^T (free dim = q).
  - h1/h2 are transposed on-chip via PE transpose (fp32 DMA transpose is not
    supported; tiles are cast to bf16 first so the transpose runs 1 cyc/row);
    weights / RoPE tables are pre-transposed on host (replicated, tiny), bf16.
  - Prologue is interleaved per 512-row slice (DMA -> transpose -> project ->
    RoPE) with per-slice SBUF tiles so nothing serializes on whole-tensor deps,
    and DMAs are emitted in true dependency order.
"""

import math
import sys

import numpy as np

for _p in ("/opt/trn_rl_repo",):
    if _p not in sys.path:
        sys.path.insert(0, _p)

import ml_dtypes

BF16 = ml_dtypes.bfloat16

S = 2048
D = 512
P = 128
B = 8
NB = S // P      # 16 key blocks of 128
DC = D // P      # 4 d-chunks of 128
EC = D // P      # 4 e-chunks (contraction for projections)
QW = 512         # tile width (free dim per matmul)
QT = S // QW     # 4 q tiles
SB = QW // P     # 4 s-blocks per q tile
NS = S // QW     # 4 s-slices for the prologue
SCALE = 1.0 / math.sqrt(D)

_compiled = None


def _build():
    import concourse.bass as bass  # noqa: F401
    import concourse.mybir as mybir
    import concourse.tile as tile
    from concourse import bacc

    f32 = mybir.dt.float32
    bf16 = mybir.dt.bfloat16
    Alu = mybir.AluOpType
    Act = mybir.ActivationFunctionType

    nc = bacc.Bacc("TRN2", target_bir_lowering=False, debug=False, num_devices=B)

    # h1/h2 arrive pre-transposed ([D, S], feature dim outer) and pre-cast to
    # bf16 on host: fp32 cannot DMA-transpose and the matmuls are bf16 anyway
    h1t_d = nc.dram_tensor("h1t", [D, S], bf16, kind="ExternalInput").ap()
    h2t_d = nc.dram_tensor("h2t", [D, S], bf16, kind="ExternalInput").ap()
    w_dram = {
        name: nc.dram_tensor(f"{name}_t", [D, D], bf16, kind="ExternalInput").ap()
        for name in ("wq", "wk", "wv", "wo")
    }
    # RoPE tables: emb = concat([freqs, freqs]) so the d>=256 half duplicates
    # the d<256 half - only half the table is shipped/stored
    cos_t = nc.dram_tensor("cos_t", [D // 2, S], bf16, kind="ExternalInput").ap()
    sin_t = nc.dram_tensor("sin_t", [D // 2, S], bf16, kind="ExternalInput").ap()
    bq_c = nc.dram_tensor("bq_c", [P, DC], f32, kind="ExternalInput").ap()
    bk_c = nc.dram_tensor("bk_c", [P, DC], f32, kind="ExternalInput").ap()
    # bo_b holds bo_eff = bo + Wo @ bv (bv folded through the value path on host)
    bo_b = nc.dram_tensor("bo_b", [P, D], f32, kind="ExternalInput").ap()
    ident_d = nc.dram_tensor("ident", [P, P], f32, kind="ExternalInput").ap()
    out = nc.dram_tensor("out", [S, D], f32, kind="ExternalOutput").ap()

    with tile.TileContext(nc) as tc:
        from contextlib import ExitStack

        with ExitStack() as ctx:
            singles = ctx.enter_context(tc.tile_pool(name="singles", bufs=1))

            def load_w(name):
                # one dma_start per weight: DMA *issue* costs ~0.7us on the
                # sequencer, so fewer+bigger transfers win at the front
                t = singles.tile([P, EC, D], bf16, tag=f"w_{name}")
                nc.sync.dma_start(
                    out=t, in_=w_dram[name].rearrange("(c p) d -> p c d", p=P)
                )
                return t

            # --- persistent tiles (DMAs emitted in dependency order) ---------
            w_sb = {}
            w_sb["wk"] = load_w("wk")
            bk_sb = singles.tile([P, DC], f32, tag="bk")
            nc.sync.dma_start(out=bk_sb, in_=bk_c)

            # per-slice persistent tensors (fine-grained dependencies)
            kt_p = [
                singles.tile([P, DC, QW], bf16, tag=f"kt{i}", name=f"kt{i}")
                for i in range(NS)
            ]
            qt_p = [
                singles.tile([P, DC, QW], bf16, tag=f"qt{i}", name=f"qt{i}")
                for i in range(NS)
            ]
            v_p = [
                singles.tile([P, SB, QW], bf16, tag=f"v{i}", name=f"v{i}")
                for i in range(NS)
            ]

            cos_sb = singles.tile([P, 2, S], bf16, tag="cos")
            sin_sb = singles.tile([P, 2, S], bf16, tag="sin")

            # ---------------- Phase A: k/v projections + RoPE ----------------
            htp = ctx.enter_context(tc.tile_pool(name="ht", bufs=3))
            ptmp = ctx.enter_context(tc.tile_pool(name="ptmp", bufs=3))
            h1t_r = h1t_d.rearrange("(c p) s -> p c s", p=P)
            h2t_r = h2t_d.rearrange("(c p) s -> p c s", p=P)

            if True:  # keep indentation structure

                def load_slice(ht_r, s2):
                    # [P, EC, QW] bf16 tile holding h^T columns [s2*QW,(s2+1)*QW)
                    ht = htp.tile([P, EC, QW], bf16, tag="ht")
                    nc.sync.dma_start(
                        out=ht, in_=ht_r[:, :, s2 * QW : (s2 + 1) * QW]
                    )
                    return ht

                def project_rope(ht, wname, b_sb, dst, s2, pp_pool, pp_tag):
                    # dst[:, dc, :] = RoPE(W @ h^T + b) for columns of slice s2
                    sl = slice(s2 * QW, (s2 + 1) * QW)
                    for pair in range(2):
                        dc0, dc2 = pair, pair + 2
                        pp = pp_pool.tile([P, 2, QW], f32, tag=pp_tag, name="pp")
                        for half, dc in ((0, dc0), (1, dc2)):
                            for ec in range(EC):
                                nc.tensor.matmul(
                                    pp[:, half, :],
                                    lhsT=w_sb[wname][:, ec, dc * P : (dc + 1) * P],
                                    rhs=ht[:, ec, :],
                                    start=(ec == 0),
                                    stop=(ec == EC - 1),
                                )
                        # rope: out[d<256] = x0*cos0 - x2*sin0
                        #       out[d>=256] = x2*cos2 + x0*sin2
                        # (bias folds into the STT's scalar add; the combines
                        # run on the otherwise-idle GpSimd engine)
                        cps = cos_sb[:, pair, sl]
                        sps = sin_sb[:, pair, sl]
                        t0 = ptmp.tile([P, QW], f32, tag="rope0")
                        nc.vector.scalar_tensor_tensor(
                            t0,
                            in0=pp[:, 0, :],
                            scalar=b_sb[:, dc0 : dc0 + 1],
                            in1=cps,
                            op0=Alu.add,
                            op1=Alu.mult,
                        )
                        t1 = ptmp.tile([P, QW], f32, tag="rope1")
                        nc.vector.scalar_tensor_tensor(
                            t1,
                            in0=pp[:, 1, :],
                            scalar=b_sb[:, dc2 : dc2 + 1],
                            in1=sps,
                            op0=Alu.add,
                            op1=Alu.mult,
                        )
                        nc.gpsimd.tensor_tensor(dst[:, dc0, :], t0, t1, Alu.subtract)
                        t2 = ptmp.tile([P, QW], f32, tag="rope0")
                        nc.vector.scalar_tensor_tensor(
                            t2,
                            in0=pp[:, 1, :],
                            scalar=b_sb[:, dc2 : dc2 + 1],
                            in1=cps,
                            op0=Alu.add,
                            op1=Alu.mult,
                        )
                        t3 = ptmp.tile([P, QW], f32, tag="rope1")
                        nc.vector.scalar_tensor_tensor(
                            t3,
                            in0=pp[:, 0, :],
                            scalar=b_sb[:, dc0 : dc0 + 1],
                            in1=sps,
                            op0=Alu.add,
                            op1=Alu.mult,
                        )
                        nc.gpsimd.tensor_tensor(dst[:, dc2, :], t2, t3, Alu.add)

                def project_v(ht, s2, psA):
                    # bv is folded into bo on host (bo_eff = bo + Wo @ bv), so
                    # this is a plain PSUM->SBUF cast on the idle ACT engine
                    for j in range(SB):
                        vp = psA.tile([P, QW], f32, tag="vp")
                        for ec in range(EC):
                            nc.tensor.matmul(
                                vp,
                                lhsT=ht[:, ec, j * P : (j + 1) * P],
                                rhs=w_sb["wv"][:, ec, :],
                                start=(ec == 0),
                                stop=(ec == EC - 1),
                            )
                        nc.scalar.copy(v_p[s2][:, j, :], vp)

            with tc.tile_pool(name="psumA", bufs=2, space="PSUM") as psA:
                cos_r = cos_t.rearrange("(c p) s -> p c s", p=P)
                sin_r = sin_t.rearrange("(c p) s -> p c s", p=P)
                for s2 in range(NS):
                    ht = load_slice(h2t_r, s2)
                    # interleave the RoPE-table chunk transfers with the h2t
                    # slices: slice RoPE pair0 needs chunks (0,2), pair1 (1,3)
                    if s2 == 0:
                        # tables issue from the (idle) gpsimd sequencer so they
                        # don't delay issue of the critical-path sync DMAs
                        nc.gpsimd.dma_start(out=cos_sb, in_=cos_r)
                        nc.gpsimd.dma_start(out=sin_sb, in_=sin_r)
                        w_sb["wv"] = load_w("wv")
                    project_rope(ht, "wk", bk_sb, kt_p[s2], s2, psA, "pp")
                    if s2 == 2:
                        # q slice 0 before the last k slice: its RoPE clears
                        # the DVE before attention needs qt_p[0]; attention
                        # only reads kt_p[3] ~25us into its kb loop
                        ht0 = load_slice(h1t_r, 0)
                        project_rope(ht0, "wq", bq_sb, qt_p[0], 0, psA, "pp")
                    project_v(ht, s2, psA)
                    if s2 == 0:
                        w_sb["wq"] = load_w("wq")
                        bq_sb = singles.tile([P, DC], f32, tag="bq")
                        nc.gpsimd.dma_start(out=bq_sb, in_=bq_c)
                        ident = singles.tile([P, P], f32, tag="ident")
                        nc.gpsimd.dma_start(out=ident, in_=ident_d)
                        ones_bf = singles.tile([P, 1], bf16, tag="ones_bf")
                        nc.vector.memset(ones_bf, 1.0)
                    if s2 == 1:
                        w_sb["wo"] = load_w("wo")
                        bo_sb = singles.tile([P, D], f32, tag="bo")
                        nc.gpsimd.dma_start(out=bo_sb, in_=bo_b)

            # ---------------- Phase B: attention -----------------------------
            # PSUM budget (8 banks): st 3 + ot 2 + cs 1 + finpp 2.
            # PV accumulates in TWO passes of 2 d-chunks each (pt tiles stay
            # resident) so that 2 banks ("finpp") are free for the NEXT q
            # slice's projection, which runs in the shadow of this q tile's
            # attention.
            with tc.tile_pool(name="ptpool", bufs=NB + 2) as ptp, tc.tile_pool(
                name="otsb", bufs=2
            ) as otp, tc.tile_pool(name="outst", bufs=3) as outp, tc.tile_pool(
                name="psum_st", bufs=3, space="PSUM"
            ) as ps_st, tc.tile_pool(
                name="psum_ot", bufs=1, space="PSUM"
            ) as ps_ot, tc.tile_pool(
                name="psum_cs", bufs=1, space="PSUM"
            ) as ps_cs, tc.tile_pool(name="psum_fin", bufs=1, space="PSUM") as ps_fin:
                for qt in range(QT):
                    if qt + 1 < QT:
                        # project+RoPE the next q slice during this attention
                        htn = load_slice(h1t_r, qt + 1)
                        project_rope(
                            htn, "wq", bq_sb, qt_p[qt + 1], qt + 1, ps_fin, "finpp"
                        )

                    cs = ps_cs.tile([1, QW], f32, tag="cs")
                    ot_sb = otp.tile([P, DC, QW], bf16, tag="ot_sb")
                    pts = []

                    # pass 1: S^T + exp + colsums + PV dc 0,1; PV(kb-1) is
                    # emitted after S^T(kb) so the PE never head-of-line
                    # blocks on exp(kb)
                    ot01 = ps_ot.tile([P, 2, QW], f32, tag="ot", name="ot01")
                    for kb in range(NB):
                        st = ps_st.tile([P, QW], f32, tag="st")
                        for dc in range(DC):
                            nc.tensor.matmul(
                                st,
                                lhsT=kt_p[kb // SB][:, dc, (kb % SB) * P : (kb % SB + 1) * P],
                                rhs=qt_p[qt][:, dc, :],
                                start=(dc == 0),
                                stop=(dc == DC - 1),
                            )
                        pt = ptp.tile([P, QW], bf16, tag="pt")
                        nc.scalar.activation(pt, st, Act.Exp, scale=SCALE)
                        pts.append(pt)
                        if kb > 0:
                            _emit_pv(nc, v_p, ones_bf, pts[kb - 1], ot01, cs, kb - 1, (0, 1))
                    _emit_pv(nc, v_p, ones_bf, pts[NB - 1], ot01, cs, NB - 1, (0, 1))
                    for dc in range(2):
                        nc.any.tensor_copy(out=ot_sb[:, dc, :], in_=ot01[:, dc, :])

                    # pass 2: PV dc 2,3 from the resident pt tiles
                    ot23 = ps_ot.tile([P, 2, QW], f32, tag="ot", name="ot23")
                    for kb in range(NB):
                        for i, dc in enumerate((2, 3)):
                            nc.tensor.matmul(
                                ot23[:, i, :],
                                lhsT=v_p[kb // SB][:, kb % SB, dc * P : (dc + 1) * P],
                                rhs=pts[kb],
                                start=(kb == 0),
                                stop=(kb == NB - 1),
                            )
                    for i, dc in enumerate((2, 3)):
                        nc.any.tensor_copy(out=ot_sb[:, dc, :], in_=ot23[:, i, :])

                    # denominators: DVE reciprocal is per-lane-serial, so get the
                    # colsums onto PARTITIONS first (4 tiny PE transposes of
                    # [1,128] slices), then reciprocal on [128,4] is ~free.
                    cs_row = outp.tile([1, QW], f32, tag="cs_row")
                    nc.any.tensor_copy(out=cs_row, in_=cs)
                    r4 = outp.tile([P, SB], f32, tag="r4")
                    for sb in range(SB):
                        tpr = ps_cs.tile([P, 1], f32, tag="cs", name="tpr")
                        nc.tensor.transpose(
                            tpr, cs_row[0:1, sb * P : (sb + 1) * P], ident[0:1, 0:1]
                        )
                        nc.any.tensor_copy(out=r4[:, sb : sb + 1], in_=tpr)
                    r4r = outp.tile([P, SB], f32, tag="r4r")
                    nc.vector.reciprocal(r4r, r4)

                    # final projection back to natural [s, d] layout; fused
                    # (fp * r) + bo in one DVE op
                    for sb in range(SB):
                        fpt = ps_fin.tile([P, 2, QW], f32, tag="finpp", name="fpt")
                        fp = fpt[:, 0, :]
                        for dc in range(DC):
                            nc.tensor.matmul(
                                fp,
                                lhsT=ot_sb[:, dc, sb * P : (sb + 1) * P],
                                rhs=w_sb["wo"][:, dc, :],
                                start=(dc == 0),
                                stop=(dc == DC - 1),
                            )
                        o_sb = outp.tile([P, D], f32, tag="ostage")
                        nc.vector.scalar_tensor_tensor(
                            o_sb,
                            in0=fp,
                            scalar=r4r[:, sb : sb + 1],
                            in1=bo_sb,
                            op0=Alu.mult,
                            op1=Alu.add,
                        )
                        row0 = (qt * SB + sb) * P
                        nc.sync.dma_start(out=out[row0 : row0 + P, :], in_=o_sb)

    nc.compile()
    return nc


def _emit_pv(nc, v_p, ones_bf, pt, ot, cs, kb, dcs):
    nc.tensor.matmul(
        cs, lhsT=ones_bf, rhs=pt, start=(kb == 0), stop=(kb == NB - 1)
    )
    for i, dc in enumerate(dcs):
        nc.tensor.matmul(
            ot[:, i, :],
            lhsT=v_p[kb // SB][:, kb % SB, dc * P : (dc + 1) * P],
            rhs=pt,
            start=(kb == 0),
            stop=(kb == NB - 1),
        )


def _get_compiled():
    global _compiled
    if _compiled is None:
        _compiled = _build()
    return _compiled


def _host_tables():
    half = D // 2
    inv_freq = 1.0 / (10000.0 ** (np.arange(half, dtype=np.float32) / half))
    t = np.arange(S, dtype=np.float32)
    freqs = np.outer(t, inv_freq)
    emb = np.concatenate([freqs, freqs], axis=-1)  # [S, D]
    # the two d-halves of emb are identical - ship only [D/2, S]
    cos_t = np.ascontiguousarray(np.cos(emb).T[: D // 2]).astype(BF16)
    sin_t = np.ascontiguousarray(np.sin(emb).T[: D // 2]).astype(BF16)
    return cos_t, sin_t


def make_in_maps(**inputs):
    cos_t, sin_t = _host_tables()
    shared = {
        "cos_t": cos_t,
        "sin_t": sin_t,
        "wq_t": np.ascontiguousarray(np.asarray(inputs["Wq"], np.float32).T).astype(BF16),
        "wk_t": np.ascontiguousarray(np.asarray(inputs["Wk"], np.float32).T).astype(BF16),
        "wv_t": np.ascontiguousarray(np.asarray(inputs["Wv"], np.float32).T).astype(BF16),
        "wo_t": np.ascontiguousarray(np.asarray(inputs["Wo"], np.float32).T).astype(BF16),
        "bq_c": np.ascontiguousarray(np.asarray(inputs["bq"], np.float32).reshape(DC, P).T),
        "bk_c": np.ascontiguousarray(np.asarray(inputs["bk"], np.float32).reshape(DC, P).T),
        # bv contributes bv @ Wo.T to every output row - fold it into bo
        "bo_b": np.ascontiguousarray(
            np.broadcast_to(
                np.asarray(inputs["bo"], np.float32)
                + np.asarray(inputs["Wo"], np.float32)
                @ np.asarray(inputs["bv"], np.float32),
                (P, D),
            )
        ),
        "ident": np.eye(P, dtype=np.float32),
    }
    h1 = np.asarray(inputs["h1"], np.float32)
    h2 = np.asarray(inputs["h2"], np.float32)
    return [
        dict(
            shared,
            h1t=np.ascontiguousarray(h1[core].T).astype(BF16),
            h2t=np.ascontiguousarray(h2[core].T).astype(BF16),
        )
        for core in range(B)
    ]


def _install_ntff_hook():
    """The agent image's antenv lacks axon_hooks; rebuild the NTFF profile hook
    from libaxon_pjrt.so (mirrors trn_agent_boot._ntff_profile_via_ctypes)."""
    try:
        from antenv.axon_hooks import get_axon_ntff_profile_hook  # noqa: F401

        return
    except ImportError:
        pass
    import contextlib
    import ctypes
    import types

    so_path = "/opt/axon/libaxon_pjrt.so"
    try:
        lib = ctypes.CDLL(so_path)
    except OSError:
        return
    if not hasattr(lib, "axon_start_nrt_profile"):
        return
    lib.axon_start_nrt_profile.argtypes = [
        ctypes.POINTER(ctypes.c_int64),
        ctypes.c_size_t,
    ]
    lib.axon_start_nrt_profile.restype = ctypes.c_int64
    lib.axon_stop_nrt_profile.argtypes = [ctypes.c_char_p]
    lib.axon_stop_nrt_profile.restype = ctypes.c_int64

    @contextlib.contextmanager
    def _hook(output_dir, device_ids):
        import jax

        jax.devices()
        if device_ids:
            ids = (ctypes.c_int64 * len(device_ids))(*device_ids)
            rc = lib.axon_start_nrt_profile(ids, len(device_ids))
        else:
            rc = lib.axon_start_nrt_profile(None, 0)
        if rc != 0:
            raise RuntimeError(f"axon_start_nrt_profile rc={rc}")
        try:
            yield
        finally:
            n = lib.axon_stop_nrt_profile(str(output_dir).encode())
            print(f"ntff profile: {n} file(s) written to {output_dir}")

    import antenv

    mod = types.ModuleType("antenv.axon_hooks")
    mod.get_axon_ntff_profile_hook = lambda: _hook
    mod.set_axon_ntff_profile_hook = lambda h: None
    sys.modules["antenv.axon_hooks"] = mod
    antenv.axon_hooks = mod


def run(trace=False, tmpdir=None, trace_cores=None, **inputs):
    from concourse.bass_utils import run_bass_kernel_spmd

    if trace:
        _install_ntff_hook()
    nc = _get_compiled()
    in_maps = make_in_maps(**inputs)
    kwargs = {}
    if tmpdir is not None:
        kwargs["tmpdir"] = tmpdir
    if trace_cores is not None:
        kwargs["trace_cores"] = trace_cores
    res = run_bass_kernel_spmd(
        nc, in_maps, core_ids=list(range(B)), trace=trace, **kwargs
    )
    out = np.stack([res.results[i]["out"] for i in range(B)]).astype(np.float32)
    return out, res


def kernel(**inputs):
    out, _ = run(trace=False, **inputs)
    return out
